# revision 1
# baseline (speedup 1.0000x reference)
"""BinLinear (sign-quantized linear) Trainium2 kernel.

Computes out = x @ sign(clip(w, -1, 1)).T for x[8192, 4096], w[4096, 4096],
distributed data-parallel over the 8 NeuronCores (each core takes 1024 rows
of x and the full weight matrix).

Per-core plan (out_shard[1024, 4096] = x_shard @ sign(w).T, contraction 4096):
  - host ships x_shard and w pre-transposed (contraction axis outer) in bf16;
    sign(bf16(w)) == sign(w) exactly, and bf16 {-1, 0, +1} are exact.
  - the x shard stays resident in SBUF (32 tiles [128, 1024] bf16, 8.4 MB).
  - w^T streams through SBUF once in [128, 512] tiles; ScalarE applies the
    Sign activation in place (sign(0) = 0, matching jnp.sign).
  - TensorE accumulates over k into 8 PSUM banks (one per 128-row group of
    the x shard), bf16 x bf16 -> fp32.
  - VectorE drains PSUM to SBUF, DMA writes fp32 output.
"""

import numpy as np
import ml_dtypes

import concourse.mybir as mybir
import concourse.tile as tile
from concourse import bacc
from concourse.bass_utils import run_bass_kernel_spmd

N_CORES = 8
N_FULL, IN_CH, OUT_CH = 8192, 4096, 4096
P = 128
OBLK = 512  # output-channel columns per PSUM bank


def build_nc(ns, in_ch, out_ch):
    """Per-core SPMD program: out[ns, out_ch] = xT.T @ sign(wT)."""
    kt = in_ch // P       # contraction tiles
    msub = ns // P        # PSUM banks in flight
    nob = out_ch // OBLK  # output-channel blocks
    assert msub <= 8 and kt * P == in_ch and nob * OBLK == out_ch

    nc = bacc.Bacc("TRN2", target_bir_lowering=False, debug=False)
    xT_d = nc.dram_tensor("xT", [in_ch, ns], mybir.dt.bfloat16, kind="ExternalInput")
    wT_d = nc.dram_tensor("wT", [in_ch, out_ch], mybir.dt.bfloat16, kind="ExternalInput")
    out_d = nc.dram_tensor("out", [ns, out_ch], mybir.dt.float32, kind="ExternalOutput")

    with tile.TileContext(nc) as tc:
        with (
            tc.tile_pool(name="xpool", bufs=1) as xpool,
            tc.tile_pool(name="wpool", bufs=16) as wpool,
            tc.tile_pool(name="wlpool", bufs=1) as wlpool,
            tc.tile_pool(name="opool", bufs=8) as opool,
            tc.tile_pool(name="const", bufs=1) as const,
            tc.tile_pool(name="pspool", bufs=1, space="PSUM") as pspool,
        ):
            zbias = const.tile([P, 1], mybir.dt.float32, name="zbias")
            nc.any.memset(zbias[:], 0.0)

            # x tiles are loaded lazily inside ob == 0's k-loop so the PE can
            # start as soon as the first (w, x) tile pair lands, instead of
            # stalling behind the full 8.4 MB x load.
            x_tiles = [None] * kt

            # w tiles for the final block are prefetched + signed during the
            # second-to-last block, so the final block can run m-outer /
            # k-inner entirely from SBUF: its 8 PSUM banks then complete
            # 6.8us apart and their drains + output DMAs hide under the
            # remaining matmuls, leaving only one bank's drain in the tail.
            wlast = [None] * kt

            for ob in range(nob - 1):
                c0 = ob * OBLK
                psums = [
                    pspool.tile([P, OBLK], mybir.dt.float32, name=f"ps_{m}")
                    for m in range(msub)
                ]
                for k in range(kt):
                    wt = wpool.tile([P, OBLK], mybir.dt.bfloat16, name="wt")
                    nc.sync.dma_start(
                        out=wt[:],
                        in_=wT_d[k * P:(k + 1) * P, c0:c0 + OBLK],
                    )
                    if x_tiles[k] is None:
                        xt = xpool.tile([P, ns], mybir.dt.bfloat16, name=f"x_{k}")
                        nc.sync.dma_start(out=xt[:], in_=xT_d[k * P:(k + 1) * P, :])
                        x_tiles[k] = xt
                    nc.scalar.activation(
                        wt[:], wt[:], mybir.ActivationFunctionType.Sign, bias=zbias[:]
                    )
                    if ob == nob - 2:
                        wl = wlpool.tile([P, OBLK], mybir.dt.bfloat16, name=f"wl_{k}")
                        nc.sync.dma_start(
                            out=wl[:],
                            in_=wT_d[k * P:(k + 1) * P, (nob - 1) * OBLK:],
                        )
                        nc.scalar.activation(
                            wl[:], wl[:], mybir.ActivationFunctionType.Sign,
                            bias=zbias[:],
                        )
                        wlast[k] = wl
                    for m in range(msub):
                        nc.tensor.matmul(
                            psums[m][:],
                            x_tiles[k][:, m * P:(m + 1) * P],
                            wt[:],
                            start=(k == 0),
                            stop=(k == kt - 1),
                        )
                for m in range(msub):
                    ot = opool.tile([P, OBLK], mybir.dt.float32, name="ot")
                    # drains stay off ScalarE: ACT is strict FIFO and shared
                    # with the sign activations, so ACT drains would stall the
                    # next block's sign -> matmul chain at block boundaries
                    # (and the Sign->Copy activation-table swap costs ~1.5us)
                    nc.vector.tensor_copy(ot[:], psums[m][:])
                    nc.sync.dma_start(
                        out=out_d[m * P:(m + 1) * P, c0:c0 + OBLK],
                        in_=ot[:],
                    )

            c0 = (nob - 1) * OBLK
            psums = [
                pspool.tile([P, OBLK], mybir.dt.float32, name=f"ps_{m}")
                for m in range(msub)
            ]
            for m in range(msub):
                for k in range(kt):
                    nc.tensor.matmul(
                        psums[m][:],
                        x_tiles[k][:, m * P:(m + 1) * P],
                        wlast[k][:],
                        start=(k == 0),
                        stop=(k == kt - 1),
                    )
                ot = opool.tile([P, OBLK], mybir.dt.float32, name="ot")
                nc.vector.tensor_copy(ot[:], psums[m][:])
                nc.sync.dma_start(
                    out=out_d[m * P:(m + 1) * P, c0:c0 + OBLK],
                    in_=ot[:],
                )
    nc.compile()
    return nc


def prep_in_maps(x, weights_real, n_cores=N_CORES):
    x = np.asarray(x, dtype=np.float32)
    weights_real = np.asarray(weights_real, dtype=np.float32)
    xT = np.ascontiguousarray(x.T).astype(ml_dtypes.bfloat16)
    wT = np.ascontiguousarray(weights_real.T).astype(ml_dtypes.bfloat16)
    ns = x.shape[0] // n_cores
    return [
        {"xT": np.ascontiguousarray(xT[:, c * ns:(c + 1) * ns]), "wT": wT}
        for c in range(n_cores)
    ]


def run(x, weights_real, trace=False, **kwargs):
    nc = build_nc(N_FULL // N_CORES, IN_CH, OUT_CH)
    in_maps = prep_in_maps(x, weights_real)
    res = run_bass_kernel_spmd(nc, in_maps, list(range(N_CORES)), trace=trace, **kwargs)
    out = np.concatenate(
        [np.asarray(res.results[c]["out"]) for c in range(N_CORES)], axis=0
    )
    return np.ascontiguousarray(out.astype(np.float32)), res


def kernel(x, weights_real):
    out, _ = run(x, weights_real)
    return out

